# revision 36
# baseline (speedup 1.0000x reference)
"""Trainium2 Bass kernel for nn_DynamicSparseConv.

Model (per sample):
    y  = mean(x, HW)                        [C]
    h  = gelu(y @ w1.T)                     [MID]
    w  = softmax((h @ w2.T).reshape(C, 9))  per-channel 3x3 kernels
    out = depthwise3x3(x, w) + x

Sharding: pure data parallel, batch 32 -> 4 samples on each of 8 cores.

Per-core design, DMA-roofline oriented (in+out = 32 MiB/core on a shared
~400 GB/s ring => ~90us floor):
  - x lands via DMA as a compact f32 tile [128, 4096] per (sample b,
    channel-block cb).  One ACT pass casts it into a zero-padded 2D bf16
    tile [128, 66*66] (interior strided 64-of-66) and produces the channel
    sums as accum_out.  Pad rows/cols are memset once at kernel start and
    never rewritten (6 persistent padded tiles).
  - conv taps read the padded tile with strided rhs APs; every tap is a
    full-rate [128, 8x64] diag matmul, no wrap-around garbage, no edge
    fixups.
  - per-sample engine budget is the DMA arrival period (22.4us), so the 9
    taps are split: 6 shifts on PE (6 x 3.4us), 2 shifts on DVE as the two
    stt merge passes over [128,1024] units (psum + tap each), and the
    center+residual ((w_c+1)*x, exact f32) on GpSimd as the final all-SBUF
    stt pass.  ACT carries casts + diag builds; every engine stays under
    the DMA period.
  - weight-gen MLP: h via K=128 matmuls on replicated w1; the 18 per-tap
    K=32 matmuls are batched into 6 K=96 matmuls (3 taps each) against a
    block-diagonal replicated-gelu rhs built with one masked multiply.
  - all 8 input DMAs are issued before any output DMA so the DMA ring
    runs ins back-to-back, then drains outs; the ring never starves.
"""

import numpy as np
from contextlib import ExitStack

import concourse.bass as bass
import concourse.tile as tile
from concourse import mybir
from concourse._compat import with_exitstack
from concourse.masks import make_identity
from concourse.bass_utils import run_bass_kernel_spmd

F32 = mybir.dt.float32
BF16 = mybir.dt.bfloat16
FP8 = mybir.dt.float8e4
AL = mybir.AluOpType
AF = mybir.ActivationFunctionType
PM = mybir.MatmulPerfMode

B, C, H, W = 32, 256, 64, 64
MID = 32
NCORES = 8
BPC = B // NCORES          # samples per core
P = 128
CB = C // P                # channel blocks
FREE = H * W               # 4096

# padded 2D bf16 layout: 66 rows x 66 cols, logical (r, s) in [-1, 64]^2
# at flat offset (r+1)*66 + (s+1)
PW = W + 2                 # 66
XPF = PW * (H + 2)         # 4356
PINT = PW + 1              # offset of x[0, 0]

NU = 4                     # merge units per pair
UROWS = H // NU            # 16 rows
UCH = UROWS * W            # 1024
CHH = 512                  # psum half-unit (one matmul group)

# all 8 shift taps run on PE as fp8e4m3 DoubleRow tap-PAIR matmuls (2 taps
# per matmul at half the cycles).  Pair k-tile strides (delta between the two
# taps' padded offsets) must be even -- delta=1 hard-faults the PE -- so the
# corners pair row-wise (delta 2) and the verticals pair together (delta 132).
# The center tap + residual ((w_c+1)*x, exact f32) is the single DVE merge.
TAP_PAIRS = [((-1, -1), (-1, 1)), ((0, -1), (0, 1)),
             ((1, -1), (1, 1)), ((-1, 0), (1, 0))]

SQRT_2_OVER_PI = 0.7978845608028654
GELU_C = 0.044715

NXP = 6                    # persistent padded tiles


def _off(r, s):
    return (r + 1) * PW + (s + 1)


@with_exitstack
def _build_body(ctx: ExitStack, tc: "tile.TileContext", x, w1t, w2l, mask3, out):
    nc = tc.nc

    consts = ctx.enter_context(tc.tile_pool(name="consts", bufs=1))
    xpool = ctx.enter_context(tc.tile_pool(name="xpool", bufs=7))
    xppool = ctx.enter_context(tc.tile_pool(name="xppool", bufs=NXP))
    opool = ctx.enter_context(tc.tile_pool(name="opool", bufs=4))
    mpool = ctx.enter_context(tc.tile_pool(name="mpool", bufs=4))
    dpool = ctx.enter_context(tc.tile_pool(name="dpool", bufs=4 * len(TAP_PAIRS)))
    cpsum = ctx.enter_context(tc.tile_pool(name="cpsum", bufs=3, space="PSUM"))
    spsum = ctx.enter_context(tc.tile_pool(name="spsum", bufs=2, space="PSUM"))

    # ---- persistent constants + padded tiles --------------------------------
    ident = consts.tile([P, P], F32)
    make_identity(nc, ident)
    w1t_sb = consts.tile([P, CB, P], BF16)
    nc.sync.dma_start(out=w1t_sb, in_=w1t.rearrange("(cb c) m -> c cb m", cb=CB))
    w2l_sb = consts.tile([P, CB, 3, P], BF16)
    nc.sync.dma_start(out=w2l_sb, in_=w2l[:, :, :, :])
    mask3_sb = consts.tile([P, 3], BF16)
    nc.sync.dma_start(out=mask3_sb, in_=mask3[:, :])
    c2 = consts.tile([P, 1], F32)
    nc.gpsimd.memset(c2, 2.0)
    c2_9 = consts.tile([P, 1], F32)
    nc.gpsimd.memset(c2_9, 2.0 / 9)

    xps = []
    for i in range(NXP):
        xp = xppool.tile([P, XPF], FP8, name=f"xp{i}", tag="xp")
        nc.gpsimd.memset(xp[:, 0:PW], 0.0)
        nc.gpsimd.memset(xp[:, XPF - PW:XPF], 0.0)
        colpad = bass.AP(tensor=xp.tensor, offset=xp.offset + PW,
                         ap=[list(xp.ap[0]), [PW, H], [W + 1, 2]])
        nc.gpsimd.memset(colpad, 0.0)
        xps.append(xp)

    st = {}  # per-sample pipeline state

    def load(b, nsplit=1):
        """Issue the input DMAs of sample b (compact f32 tiles)."""
        xts = []
        seg = FREE // nsplit
        for cb in range(CB):
            xt = xpool.tile([P, FREE], F32, name=f"xt{b}_{cb}", tag="xt")
            xsrc = x[b, cb * P:(cb + 1) * P].rearrange("c h w -> c (h w)")
            for j in range(nsplit):
                sl = slice(j * seg, (j + 1) * seg)
                nc.sync.dma_start(out=xt[:, sl], in_=xsrc[:, sl])
            xts.append(xt)
        st[b] = {"xts": xts, "nsplit": nsplit}

    def cast(b):
        """ACT pass per cb: f32 -> bf16 into the padded tile + channel sums."""
        nsplit = st[b]["nsplit"]
        sums = mpool.tile([P, CB * nsplit], F32, name=f"sums{b}", tag="sums")
        rpc = H // nsplit
        for cb in range(CB):
            xt = st[b]["xts"][cb]
            xp = xps[(b * CB + cb) % NXP]
            for j in range(nsplit):
                interior = bass.AP(
                    tensor=xp.tensor,
                    offset=xp.offset + PINT + j * rpc * PW,
                    ap=[list(xp.ap[0]), [PW, rpc], [1, W]],
                )
                nc.scalar.activation(
                    out=interior, in_=xt[:, j * rpc * W:(j + 1) * rpc * W],
                    func=AF.Copy,
                    accum_out=sums[:, cb * nsplit + j:cb * nsplit + j + 1],
                )
        st[b]["sums"] = sums

    def prep_h(b):
        """h-matmul + gelu chain for sample b (PE slot: one tiny matmul)."""
        sums = st[b]["sums"]
        nsplit = st[b]["nsplit"]
        ncols = CB * nsplit
        # the whole scalar chain runs on ACT: the DVE queue is saturated with
        # merges, and threading prep through it would stall the next conv
        sums_bf = mpool.tile([P, ncols], BF16, name=f"sums_bf{b}", tag="sums_bf")
        # fold the 1/(H*W) of the mean in here, so hps comes out as u directly
        nc.scalar.mul(sums_bf, sums, 1.0 / FREE)
        hps = spsum.tile([P, 9], F32, name=f"hps{b}", tag="sps")
        for j in range(ncols):
            nc.tensor.matmul(
                hps[:, 0:1], lhsT=w1t_sb[:, j // nsplit, :],
                rhs=sums_bf[:, j:j + 1],
                start=(j == 0), stop=(j == ncols - 1),
            )
        u = mpool.tile([P, 1], F32, name=f"u{b}", tag="u")
        nc.scalar.copy(u, hps[:, 0:1])
        # tanh-based gelu: g = u*(1+tanh(sqrt(2/pi)*(u + 0.044715 u^3)))
        # (the usual 0.5 is folded into w2l on the host)
        sq = mpool.tile([P, 1], F32, name=f"sq{b}", tag="sq")
        nc.scalar.mul(sq, u, u)
        c1 = mpool.tile([P, 1], F32, name=f"c1{b}", tag="c1")
        nc.scalar.activation(c1, sq, AF.Identity, bias=1.0, scale=GELU_C)
        arg = mpool.tile([P, 1], F32, name=f"arg{b}", tag="arg")
        nc.scalar.mul(arg, u, c1)
        th = mpool.tile([P, 1], F32, name=f"th{b}", tag="th")
        nc.scalar.activation(th, arg, AF.Tanh, scale=SQRT_2_OVER_PI)
        g4 = mpool.tile([P, 1], F32, name=f"g4{b}", tag="g4")
        # g = (th + 1) * u  =  th*u + u; g is 4x-replicated along partitions
        # because u is (w1 host-tiled 4x)
        nc.scalar.activation(g4, th, AF.Identity, bias=u, scale=u)
        # block-diagonal gelu rhs [96, 3]: rows 32j+m of col j hold g[m]
        rg = mpool.tile([P, 3], BF16, name=f"rg{b}", tag="rg")
        nc.scalar.mul(rg, mask3_sb, g4)
        st[b]["rg"] = rg

    def prep_w(b):
        """wgen matmuls -> softmax -> diag matrices for sample b."""
        rg = st[b]["rg"]
        # 6 batched wgen matmuls: K=96 (3 taps x 32 mid), free=3
        wgs = [spsum.tile([P, 9], F32, name=f"wg{b}_{cb}", tag="sps")
               for cb in range(CB)]
        for cb in range(CB):
            for g in range(3):
                nc.tensor.matmul(
                    wgs[cb][:, 3 * g:3 * g + 3],
                    lhsT=w2l_sb[0:3 * MID, cb, g, :],
                    rhs=rg[0:3 * MID, :],
                    start=True, stop=True,
                )

        st[b]["smw"] = []
        st[b]["wc1"] = []
        st[b]["diags"] = []
        for cb in range(CB):
            ew = mpool.tile([P, 9], F32, name=f"ew{b}_{cb}", tag="ew")
            den = mpool.tile([P, 1], F32, name=f"den{b}_{cb}", tag="den")
            nc.scalar.activation(ew, wgs[cb], AF.Exp, accum_out=den)
            # 1/den via 2 Newton steps from x0=1/9 entirely on ACT (den is
            # within ~8% of 9, so rel err <= 4e-5; keeps the DVE queue free
            # of a long-latency op that would head-of-line block the merges):
            #   e1  = 2 - den/9            (= (2 - d*x0) / 1)
            #   y   = den * e1             (= 9 * d * x1)
            #   e2p = 2/9 - y/81           (= (2 - d*x1) / 9)
            #   smw = (ew * e1) * e2p      (= ew * x1 * (2 - d*x1))
            e1 = mpool.tile([P, 1], F32, name=f"e1{b}_{cb}", tag="e1")
            nc.scalar.activation(e1, den, AF.Identity, bias=c2, scale=-1.0 / 9)
            y = mpool.tile([P, 1], F32, name=f"y{b}_{cb}", tag="y")
            nc.scalar.mul(y, den, e1)
            e2p = mpool.tile([P, 1], F32, name=f"e2p{b}_{cb}", tag="e2p")
            nc.scalar.activation(
                e2p, y, AF.Identity, bias=c2_9, scale=-1.0 / 81)
            m1 = mpool.tile([P, 9], F32, name=f"m1{b}_{cb}", tag="m1")
            nc.scalar.mul(m1, ew, e1)
            smw = mpool.tile([P, 9], F32, name=f"smw{b}_{cb}", tag="smw")
            nc.scalar.mul(smw, m1, e2p)
            # merge coefficient: w_center + 1 (center tap fused with residual)
            wc1 = mpool.tile([P, 1], F32, name=f"wc1{b}_{cb}", tag="wc1")
            nc.scalar.add(wc1, smw[:, 4:5], 1.0)

            # DoubleRow tap-pair diagonals [P, 2, P] fp8; built on DVE (it is
            # nearly idle now -- merges are a single pass per unit)
            diags = []
            for k, (t1, t2) in enumerate(TAP_PAIRS):
                dg = dpool.tile([P, 2, P], FP8, name=f"dg{b}_{cb}_{k}", tag="dg")
                for j, (r, s) in enumerate((t1, t2)):
                    tcol = (r + 1) * 3 + (s + 1)
                    nc.vector.tensor_scalar_mul(
                        dg[:, j, :], ident, smw[:, tcol:tcol + 1])
                diags.append(dg)
            st[b]["smw"].append(smw)
            st[b]["wc1"].append(wc1)
            st[b]["diags"].append(diags)

    def conv_cb(b, cb, ts=None):
        """Depthwise conv + merges + output DMA for (sample b, block cb)."""
        with ExitStack() as cvs:
            xp = xps[(b * CB + cb) % NXP]
            smw = st[b]["smw"][cb]
            wc1 = st[b]["wc1"][cb]
            diags = st[b]["diags"][cb]
            xt = st[b]["xts"][cb]
            for u in range(NU):
                if ts is not None:
                    cvs.enter_context(tc.tile_wait_until((ts + u) / 1000.0))
                ps = cpsum.tile([P, UCH], F32, name=f"ps{b}_{cb}_{u}", tag="ps")
                for half in range(2):
                    r0 = u * UROWS + half * 8
                    for k, (t1, t2) in enumerate(TAP_PAIRS):
                        delta = _off(*t2) - _off(*t1)
                        rhs = bass.AP(
                            tensor=xp.tensor,
                            offset=xp.offset + _off(*t1) + r0 * PW,
                            ap=[list(xp.ap[0]), [delta, 2], [PW, 8], [1, W]],
                        )
                        nc.tensor.matmul(
                            ps[:, half * CHH:(half + 1) * CHH],
                            lhsT=diags[k][:, :, :],
                            rhs=rhs,
                            start=(k == 0), stop=(k == len(TAP_PAIRS) - 1),
                            perf_mode=PM.DoubleRow,
                        )
                ot = opool.tile([P, UCH], F32, name=f"ot{b}_{cb}_{u}", tag="ot")
                # single merge (DVE): ot = (w_c + 1) * x + psum(8 taps); the
                # residual reads the exact f32 input
                nc.vector.scalar_tensor_tensor(
                    out=ot, in0=xt[:, u * UCH:(u + 1) * UCH], scalar=wc1,
                    in1=ps, op0=AL.mult, op1=AL.add,
                )
                nc.sync.dma_start(
                    out=out[b, cb * P:(cb + 1) * P,
                            u * UROWS:(u + 1) * UROWS].rearrange(
                        "c h w -> c (h w)"),
                    in_=ot,
                )
        if cb == CB - 1:
            del st[b]

    # ---- emission ----------------------------------------------------------
    # All input DMAs are issued first so the DMA ring runs the four samples
    # back-to-back.  Every phase is stamped with its ideal-schedule time via
    # tile_wait_until -- the Tile list-scheduler orders each engine queue by
    # these floors, which kills the head-of-line convoys that a misestimated
    # dependency (e.g. next sample's diag builds before this sample's merges
    # on the DVE queue) would otherwise cause.
    arr = [20.0 + 11.2 * b for b in range(BPC)]   # us, DMA(b) complete
    tcv = [24.0, 40.0, 55.0, 70.0]                # us, conv(b) start

    def at(us):
        return tc.tile_wait_until(us / 1000.0)

    load(0, nsplit=2)
    load(1, nsplit=2)
    load(2, nsplit=2)
    load(3, nsplit=2)
    with at(11.0):
        cast(0)
    with at(21.0):
        prep_h(0)
    with at(23.0):
        prep_w(0)
    with at(arr[1] - 5):
        cast(1)
    conv_cb(0, 0, ts=tcv[0])
    with at(arr[1] + 2):
        prep_h(1)
    with at(arr[1] + 4.5):
        prep_w(1)
    conv_cb(0, 1, ts=tcv[0] + 4)
    with at(arr[2] - 5):
        cast(2)
    with at(arr[2] + 2):
        prep_h(2)
    with at(arr[2] + 4.5):
        prep_w(2)
    conv_cb(1, 0, ts=tcv[1])
    conv_cb(1, 1, ts=tcv[1] + 4)
    with at(arr[3] - 5):
        cast(3)
    with at(arr[3] + 2):
        prep_h(3)
    with at(arr[3] + 4.5):
        prep_w(3)
    conv_cb(2, 0, ts=tcv[2])
    conv_cb(2, 1, ts=tcv[2] + 4)
    conv_cb(3, 0, ts=tcv[3])
    conv_cb(3, 1, ts=tcv[3] + 4)


def build_nc():
    nc = bass.Bass(trn_type="TRN2")
    x = nc.dram_tensor("x", [BPC, C, H, W], F32, kind="ExternalInput")
    w1t = nc.dram_tensor("w1t", [C, P], BF16, kind="ExternalInput")
    w2l = nc.dram_tensor("w2l", [P, CB, 3, P], BF16, kind="ExternalInput")
    mask3 = nc.dram_tensor("mask3", [P, 3], BF16, kind="ExternalInput")
    out = nc.dram_tensor("out", [BPC, C, H, W], F32, kind="ExternalOutput")
    with tile.TileContext(nc) as tc:
        _build_body(tc, x, w1t, w2l, mask3, out)
    return nc


def host_prep(w1: np.ndarray, w2: np.ndarray):
    """Layout/dtype-only prep of the (tiny) shared weights."""
    import ml_dtypes

    w1t = np.ascontiguousarray(np.asarray(w1, dtype=np.float32).T)  # [C, MID]
    # 4x-replicated along mid so h comes out replicated across partitions
    w1t4 = np.tile(w1t, (1, 4))  # [C, 4*MID]

    # w2 rows are r = c*9 + t; batched-K layout, pre-scaled by 0.5
    # w2l[m + 32*j, cb, g, c] = 0.5 * w2[(cb*128+c)*9 + 3g+j, m]
    w2r = np.asarray(w2, dtype=np.float32).reshape(CB, P, 3, 3, MID) * 0.5
    w2l = np.zeros((P, CB, 3, P), dtype=np.float32)
    for j in range(3):
        # [cb, c, g, m] -> [m, cb, g, c]
        w2l[32 * j:32 * (j + 1)] = w2r[:, :, :, j, :].transpose(3, 0, 2, 1)

    mask3 = np.zeros((P, 3), dtype=np.float32)
    for j in range(3):
        mask3[32 * j:32 * (j + 1), j] = 1.0

    return (w1t4.astype(ml_dtypes.bfloat16), w2l.astype(ml_dtypes.bfloat16),
            mask3.astype(ml_dtypes.bfloat16))


# TPB instructions have a single EVENTS (wait) slot and this walrus refuses
# >1 sync-wait on them (Matmult, TensorScalarPtr, DMACopy, ...).  Drain is
# Tile's standard multi-wait tail barrier, which walrus does handle.
_SPLIT_WAIT_SKIP = {"EventSemaphore"}


def _split_matmul_waits_json(data: bytes) -> bytes:
    """Move excess sync-waits on single-wait-slot instructions onto
    EventSemaphore instructions inserted immediately before them on the same
    engine queue (semantically identical)."""
    import orjson

    m = orjson.loads(data)
    cnt = 0
    for fn in m.get("functions", []):
        for bb in fn.get("blocks", []):
            insts = bb.get("instructions")
            if not insts:
                continue
            out = []
            changed = False
            for ins in insts:
                # walrus requires 5-dim input APs on Pool; bass emits the raw
                # (optimized) AP, so pad singleton dims after the partition dim
                if ins.get("opcode") == "Pool":
                    for a in ins.get("ins", []):
                        ap = a.get("ap")
                        if ap is not None and len(ap) < 5:
                            pad = [[1, 1]] * (5 - len(ap))
                            a["ap"] = [ap[0]] + pad + list(ap[1:])
                            changed = True
                si = ins.get("sync_info")
                if (
                    ins.get("opcode") not in _SPLIT_WAIT_SKIP
                    and si
                    and len(si.get("on_wait") or []) > 1
                ):
                    waits = si["on_wait"]
                    for w in waits[:-1]:
                        out.append({
                            "name": f"EVW-{cnt}",
                            "opcode": "EventSemaphore",
                            "engine": ins["engine"],
                            "ins": [],
                            "outs": [],
                            "debug": ins.get("debug", 0),
                            "sync_info": {"on_wait": [w], "on_update": []},
                        })
                        cnt += 1
                    si["on_wait"] = [waits[-1]]
                    changed = True
                out.append(ins)
            if changed:
                bb["instructions"] = out
    return orjson.dumps(m)


_CACHE: dict = {}


def _get_nc():
    if "nc" not in _CACHE:
        nc = build_nc()
        orig = nc.to_json_bytes
        nc.to_json_bytes = lambda: _split_matmul_waits_json(orig())
        _CACHE["nc"] = nc
    return _CACHE["nc"]


def kernel(x, w1, w2, trace: bool = False, **run_kwargs):
    x = np.ascontiguousarray(np.asarray(x, dtype=np.float32))
    assert x.shape == (B, C, H, W)
    w1t, w2l, mask3 = host_prep(w1, w2)

    nc = _get_nc()
    in_maps = [
        {"x": x[i * BPC:(i + 1) * BPC], "w1t": w1t, "w2l": w2l, "mask3": mask3}
        for i in range(NCORES)
    ]
    res = run_bass_kernel_spmd(
        nc, in_maps, core_ids=list(range(NCORES)), trace=trace, **run_kwargs
    )
    _CACHE["last_results"] = res
    out = np.concatenate([res.results[i]["out"] for i in range(NCORES)], axis=0)
    return out


# revision 38
# speedup vs baseline: 1.1420x; 1.1420x over previous
"""Trainium2 Bass kernel for nn_DynamicSparseConv.

Model (per sample):
    y  = mean(x, HW)                        [C]
    h  = gelu(y @ w1.T)                     [MID]
    w  = softmax((h @ w2.T).reshape(C, 9))  per-channel 3x3 kernels
    out = depthwise3x3(x, w) + x

Sharding: pure data parallel, batch 32 -> 4 samples on each of 8 cores.

Per-core design, DMA-roofline oriented (in+out = 32 MiB/core on a shared
~400 GB/s ring => ~90us floor):
  - x lands via DMA as a compact f32 tile [128, 4096] per (sample b,
    channel-block cb).  One ACT pass casts it into a zero-padded 2D bf16
    tile [128, 66*66] (interior strided 64-of-66) and produces the channel
    sums as accum_out.  Pad rows/cols are memset once at kernel start and
    never rewritten (6 persistent padded tiles).
  - conv taps read the padded tile with strided rhs APs; every tap is a
    full-rate [128, 8x64] diag matmul, no wrap-around garbage, no edge
    fixups.
  - per-sample engine budget is the DMA arrival period (22.4us), so the 9
    taps are split: 6 shifts on PE (6 x 3.4us), 2 shifts on DVE as the two
    stt merge passes over [128,1024] units (psum + tap each), and the
    center+residual ((w_c+1)*x, exact f32) on GpSimd as the final all-SBUF
    stt pass.  ACT carries casts + diag builds; every engine stays under
    the DMA period.
  - weight-gen MLP: h via K=128 matmuls on replicated w1; the 18 per-tap
    K=32 matmuls are batched into 6 K=96 matmuls (3 taps each) against a
    block-diagonal replicated-gelu rhs built with one masked multiply.
  - all 8 input DMAs are issued before any output DMA so the DMA ring
    runs ins back-to-back, then drains outs; the ring never starves.
"""

import numpy as np
from contextlib import ExitStack

import concourse.bass as bass
import concourse.tile as tile
from concourse import mybir
from concourse._compat import with_exitstack
from concourse.masks import make_identity
from concourse.bass_utils import run_bass_kernel_spmd

F32 = mybir.dt.float32
BF16 = mybir.dt.bfloat16
FP8 = mybir.dt.float8e4
AL = mybir.AluOpType
AF = mybir.ActivationFunctionType
PM = mybir.MatmulPerfMode

B, C, H, W = 32, 256, 64, 64
MID = 32
NCORES = 8
BPC = B // NCORES          # samples per core
P = 128
CB = C // P                # channel blocks
FREE = H * W               # 4096

# padded 2D bf16 layout: 66 rows x 66 cols, logical (r, s) in [-1, 64]^2
# at flat offset (r+1)*66 + (s+1)
PW = W + 2                 # 66
XPF = PW * (H + 2)         # 4356
PINT = PW + 1              # offset of x[0, 0]

NU = 4                     # merge units per pair
UROWS = H // NU            # 16 rows
UCH = UROWS * W            # 1024
CHH = 512                  # psum half-unit (one matmul group)

# all 8 shift taps run on PE as fp8e4m3 DoubleRow tap-PAIR matmuls (2 taps
# per matmul at half the cycles).  Pair k-tile strides (delta between the two
# taps' padded offsets) must be even -- delta=1 hard-faults the PE -- so the
# corners pair row-wise (delta 2) and the verticals pair together (delta 132).
# The center tap + residual ((w_c+1)*x, exact f32) is the single DVE merge.
TAP_PAIRS = [((-1, -1), (-1, 1)), ((0, -1), (0, 1)),
             ((1, -1), (1, 1)), ((-1, 0), (1, 0))]

SQRT_2_OVER_PI = 0.7978845608028654
GELU_C = 0.044715

NXP = 6                    # persistent padded tiles


def _off(r, s):
    return (r + 1) * PW + (s + 1)


@with_exitstack
def _build_body(ctx: ExitStack, tc: "tile.TileContext", x, w1t, w2l, mask3, out):
    nc = tc.nc

    consts = ctx.enter_context(tc.tile_pool(name="consts", bufs=1))
    xpool = ctx.enter_context(tc.tile_pool(name="xpool", bufs=7))
    xppool = ctx.enter_context(tc.tile_pool(name="xppool", bufs=NXP))
    opool = ctx.enter_context(tc.tile_pool(name="opool", bufs=4))
    mpool = ctx.enter_context(tc.tile_pool(name="mpool", bufs=4))
    dpool = ctx.enter_context(tc.tile_pool(name="dpool", bufs=4 * len(TAP_PAIRS)))
    cpsum = ctx.enter_context(tc.tile_pool(name="cpsum", bufs=3, space="PSUM"))
    spsum = ctx.enter_context(tc.tile_pool(name="spsum", bufs=2, space="PSUM"))

    # ---- persistent constants + padded tiles --------------------------------
    ident = consts.tile([P, P], F32)
    make_identity(nc, ident)
    w1t_sb = consts.tile([P, CB, P], BF16)
    nc.sync.dma_start(out=w1t_sb, in_=w1t.rearrange("(cb c) m -> c cb m", cb=CB))
    w2l_sb = consts.tile([P, CB, 3, P], BF16)
    nc.sync.dma_start(out=w2l_sb, in_=w2l[:, :, :, :])
    mask3_sb = consts.tile([P, 3], BF16)
    nc.sync.dma_start(out=mask3_sb, in_=mask3[:, :])
    c2 = consts.tile([P, 1], F32)
    nc.gpsimd.memset(c2, 2.0)
    c2_9 = consts.tile([P, 1], F32)
    nc.gpsimd.memset(c2_9, 2.0 / 9)

    xps = []
    for i in range(NXP):
        xp = xppool.tile([P, XPF], FP8, name=f"xp{i}", tag="xp")
        nc.gpsimd.memset(xp[:, 0:PW], 0.0)
        nc.gpsimd.memset(xp[:, XPF - PW:XPF], 0.0)
        colpad = bass.AP(tensor=xp.tensor, offset=xp.offset + PW,
                         ap=[list(xp.ap[0]), [PW, H], [W + 1, 2]])
        nc.gpsimd.memset(colpad, 0.0)
        xps.append(xp)

    st = {}  # per-sample pipeline state

    def load(b, nsplit=1):
        """Issue the input DMAs of sample b (compact f32 tiles)."""
        xts = []
        seg = FREE // nsplit
        for cb in range(CB):
            xt = xpool.tile([P, FREE], F32, name=f"xt{b}_{cb}", tag="xt")
            xsrc = x[b, cb * P:(cb + 1) * P].rearrange("c h w -> c (h w)")
            for j in range(nsplit):
                sl = slice(j * seg, (j + 1) * seg)
                nc.sync.dma_start(out=xt[:, sl], in_=xsrc[:, sl])
            xts.append(xt)
        st[b] = {"xts": xts, "nsplit": nsplit}

    def cast(b):
        """ACT pass per cb: f32 -> bf16 into the padded tile + channel sums."""
        nsplit = st[b]["nsplit"]
        sums = mpool.tile([P, CB * nsplit], F32, name=f"sums{b}", tag="sums")
        rpc = H // nsplit
        for cb in range(CB):
            xt = st[b]["xts"][cb]
            xp = xps[(b * CB + cb) % NXP]
            for j in range(nsplit):
                interior = bass.AP(
                    tensor=xp.tensor,
                    offset=xp.offset + PINT + j * rpc * PW,
                    ap=[list(xp.ap[0]), [PW, rpc], [1, W]],
                )
                nc.scalar.activation(
                    out=interior, in_=xt[:, j * rpc * W:(j + 1) * rpc * W],
                    func=AF.Copy,
                    accum_out=sums[:, cb * nsplit + j:cb * nsplit + j + 1],
                )
        st[b]["sums"] = sums

    def prep_h(b):
        """h-matmul + gelu chain for sample b (PE slot: one tiny matmul)."""
        sums = st[b]["sums"]
        nsplit = st[b]["nsplit"]
        ncols = CB * nsplit
        # the whole scalar chain runs on ACT: the DVE queue is saturated with
        # merges, and threading prep through it would stall the next conv
        sums_bf = mpool.tile([P, ncols], BF16, name=f"sums_bf{b}", tag="sums_bf")
        # fold the 1/(H*W) of the mean in here, so hps comes out as u directly
        nc.scalar.mul(sums_bf, sums, 1.0 / FREE)
        hps = spsum.tile([P, 9], F32, name=f"hps{b}", tag="sps")
        for j in range(ncols):
            nc.tensor.matmul(
                hps[:, 0:1], lhsT=w1t_sb[:, j // nsplit, :],
                rhs=sums_bf[:, j:j + 1],
                start=(j == 0), stop=(j == ncols - 1),
            )
        u = mpool.tile([P, 1], F32, name=f"u{b}", tag="u")
        nc.scalar.copy(u, hps[:, 0:1])
        # tanh-based gelu: g = u*(1+tanh(sqrt(2/pi)*(u + 0.044715 u^3)))
        # (the usual 0.5 is folded into w2l on the host)
        sq = mpool.tile([P, 1], F32, name=f"sq{b}", tag="sq")
        nc.scalar.mul(sq, u, u)
        c1 = mpool.tile([P, 1], F32, name=f"c1{b}", tag="c1")
        nc.scalar.activation(c1, sq, AF.Identity, bias=1.0, scale=GELU_C)
        arg = mpool.tile([P, 1], F32, name=f"arg{b}", tag="arg")
        nc.scalar.mul(arg, u, c1)
        th = mpool.tile([P, 1], F32, name=f"th{b}", tag="th")
        nc.scalar.activation(th, arg, AF.Tanh, scale=SQRT_2_OVER_PI)
        g4 = mpool.tile([P, 1], F32, name=f"g4{b}", tag="g4")
        # g = (th + 1) * u  =  th*u + u; g is 4x-replicated along partitions
        # because u is (w1 host-tiled 4x)
        nc.scalar.activation(g4, th, AF.Identity, bias=u, scale=u)
        # block-diagonal gelu rhs [96, 3]: rows 32j+m of col j hold g[m]
        rg = mpool.tile([P, 3], BF16, name=f"rg{b}", tag="rg")
        nc.scalar.mul(rg, mask3_sb, g4)
        st[b]["rg"] = rg

    def prep_w(b):
        """wgen matmuls -> softmax -> diag matrices for sample b."""
        rg = st[b]["rg"]
        # 6 batched wgen matmuls: K=96 (3 taps x 32 mid), free=3
        wgs = [spsum.tile([P, 9], F32, name=f"wg{b}_{cb}", tag="sps")
               for cb in range(CB)]
        for cb in range(CB):
            for g in range(3):
                nc.tensor.matmul(
                    wgs[cb][:, 3 * g:3 * g + 3],
                    lhsT=w2l_sb[0:3 * MID, cb, g, :],
                    rhs=rg[0:3 * MID, :],
                    start=True, stop=True,
                )

        st[b]["smw"] = []
        st[b]["wc1"] = []
        st[b]["diags"] = []
        for cb in range(CB):
            ew = mpool.tile([P, 9], F32, name=f"ew{b}_{cb}", tag="ew")
            den = mpool.tile([P, 1], F32, name=f"den{b}_{cb}", tag="den")
            nc.scalar.activation(ew, wgs[cb], AF.Exp, accum_out=den)
            # 1/den via 2 Newton steps from x0=1/9 entirely on ACT (den is
            # within ~8% of 9, so rel err <= 4e-5; keeps the DVE queue free
            # of a long-latency op that would head-of-line block the merges):
            #   e1  = 2 - den/9            (= (2 - d*x0) / 1)
            #   y   = den * e1             (= 9 * d * x1)
            #   e2p = 2/9 - y/81           (= (2 - d*x1) / 9)
            #   smw = (ew * e1) * e2p      (= ew * x1 * (2 - d*x1))
            e1 = mpool.tile([P, 1], F32, name=f"e1{b}_{cb}", tag="e1")
            nc.scalar.activation(e1, den, AF.Identity, bias=c2, scale=-1.0 / 9)
            y = mpool.tile([P, 1], F32, name=f"y{b}_{cb}", tag="y")
            nc.scalar.mul(y, den, e1)
            e2p = mpool.tile([P, 1], F32, name=f"e2p{b}_{cb}", tag="e2p")
            nc.scalar.activation(
                e2p, y, AF.Identity, bias=c2_9, scale=-1.0 / 81)
            m1 = mpool.tile([P, 9], F32, name=f"m1{b}_{cb}", tag="m1")
            nc.scalar.mul(m1, ew, e1)
            smw = mpool.tile([P, 9], F32, name=f"smw{b}_{cb}", tag="smw")
            nc.scalar.mul(smw, m1, e2p)
            # merge coefficient: w_center + 1 (center tap fused with residual)
            wc1 = mpool.tile([P, 1], F32, name=f"wc1{b}_{cb}", tag="wc1")
            nc.scalar.add(wc1, smw[:, 4:5], 1.0)

            # DoubleRow tap-pair diagonals [P, 2, P] fp8; built on DVE (it is
            # nearly idle now -- merges are a single pass per unit)
            diags = []
            for k, (t1, t2) in enumerate(TAP_PAIRS):
                dg = dpool.tile([P, 2, P], FP8, name=f"dg{b}_{cb}_{k}", tag="dg")
                for j, (r, s) in enumerate((t1, t2)):
                    tcol = (r + 1) * 3 + (s + 1)
                    nc.vector.tensor_scalar_mul(
                        dg[:, j, :], ident, smw[:, tcol:tcol + 1])
                diags.append(dg)
            st[b]["smw"].append(smw)
            st[b]["wc1"].append(wc1)
            st[b]["diags"].append(diags)

    def conv_cb(b, cb, ts=None):
        """Depthwise conv + merges + output DMA for (sample b, block cb)."""
        with ExitStack() as cvs:
            xp = xps[(b * CB + cb) % NXP]
            smw = st[b]["smw"][cb]
            wc1 = st[b]["wc1"][cb]
            diags = st[b]["diags"][cb]
            xt = st[b]["xts"][cb]
            for u in range(NU):
                ps = cpsum.tile([P, UCH], F32, name=f"ps{b}_{cb}_{u}", tag="ps")
                for half in range(2):
                    r0 = u * UROWS + half * 8
                    for k, (t1, t2) in enumerate(TAP_PAIRS):
                        delta = _off(*t2) - _off(*t1)
                        rhs = bass.AP(
                            tensor=xp.tensor,
                            offset=xp.offset + _off(*t1) + r0 * PW,
                            ap=[list(xp.ap[0]), [delta, 2], [PW, 8], [1, W]],
                        )
                        nc.tensor.matmul(
                            ps[:, half * CHH:(half + 1) * CHH],
                            lhsT=diags[k][:, :, :],
                            rhs=rhs,
                            start=(k == 0), stop=(k == len(TAP_PAIRS) - 1),
                            perf_mode=PM.DoubleRow,
                        )
                ot = opool.tile([P, UCH], F32, name=f"ot{b}_{cb}_{u}", tag="ot")
                # single merge (DVE): ot = (w_c + 1) * x + psum(8 taps); the
                # residual reads the exact f32 input
                nc.vector.scalar_tensor_tensor(
                    out=ot, in0=xt[:, u * UCH:(u + 1) * UCH], scalar=wc1,
                    in1=ps, op0=AL.mult, op1=AL.add,
                )
                nc.sync.dma_start(
                    out=out[b, cb * P:(cb + 1) * P,
                            u * UROWS:(u + 1) * UROWS].rearrange(
                        "c h w -> c (h w)"),
                    in_=ot,
                )
        if cb == CB - 1:
            del st[b]

    # ---- emission ----------------------------------------------------------
    # All input DMAs are issued first so the DMA ring runs the four samples
    # back-to-back.  Every phase is stamped with its ideal-schedule time via
    # tile_wait_until -- the Tile list-scheduler orders each engine queue by
    # these floors, which kills the head-of-line convoys that a misestimated
    # dependency (e.g. next sample's diag builds before this sample's merges
    # on the DVE queue) would otherwise cause.
    load(0, nsplit=2)
    load(1, nsplit=2)
    load(2, nsplit=2)
    load(3, nsplit=2)
    cast(0)
    prep_h(0)
    prep_w(0)
    cast(1)
    conv_cb(0, 0)
    prep_h(1)
    prep_w(1)
    conv_cb(0, 1)
    cast(2)
    prep_h(2)
    conv_cb(1, 0)
    prep_w(2)
    conv_cb(1, 1)
    cast(3)
    prep_h(3)
    conv_cb(2, 0)
    prep_w(3)
    conv_cb(2, 1)
    conv_cb(3, 0)
    conv_cb(3, 1)


def build_nc():
    nc = bass.Bass(trn_type="TRN2")
    x = nc.dram_tensor("x", [BPC, C, H, W], F32, kind="ExternalInput")
    w1t = nc.dram_tensor("w1t", [C, P], BF16, kind="ExternalInput")
    w2l = nc.dram_tensor("w2l", [P, CB, 3, P], BF16, kind="ExternalInput")
    mask3 = nc.dram_tensor("mask3", [P, 3], BF16, kind="ExternalInput")
    out = nc.dram_tensor("out", [BPC, C, H, W], F32, kind="ExternalOutput")
    with tile.TileContext(nc) as tc:
        _build_body(tc, x, w1t, w2l, mask3, out)
    return nc


def host_prep(w1: np.ndarray, w2: np.ndarray):
    """Layout/dtype-only prep of the (tiny) shared weights."""
    import ml_dtypes

    w1t = np.ascontiguousarray(np.asarray(w1, dtype=np.float32).T)  # [C, MID]
    # 4x-replicated along mid so h comes out replicated across partitions
    w1t4 = np.tile(w1t, (1, 4))  # [C, 4*MID]

    # w2 rows are r = c*9 + t; batched-K layout, pre-scaled by 0.5
    # w2l[m + 32*j, cb, g, c] = 0.5 * w2[(cb*128+c)*9 + 3g+j, m]
    w2r = np.asarray(w2, dtype=np.float32).reshape(CB, P, 3, 3, MID) * 0.5
    w2l = np.zeros((P, CB, 3, P), dtype=np.float32)
    for j in range(3):
        # [cb, c, g, m] -> [m, cb, g, c]
        w2l[32 * j:32 * (j + 1)] = w2r[:, :, :, j, :].transpose(3, 0, 2, 1)

    mask3 = np.zeros((P, 3), dtype=np.float32)
    for j in range(3):
        mask3[32 * j:32 * (j + 1), j] = 1.0

    return (w1t4.astype(ml_dtypes.bfloat16), w2l.astype(ml_dtypes.bfloat16),
            mask3.astype(ml_dtypes.bfloat16))


# TPB instructions have a single EVENTS (wait) slot and this walrus refuses
# >1 sync-wait on them (Matmult, TensorScalarPtr, DMACopy, ...).  Drain is
# Tile's standard multi-wait tail barrier, which walrus does handle.
_SPLIT_WAIT_SKIP = {"EventSemaphore"}


def _split_matmul_waits_json(data: bytes) -> bytes:
    """Move excess sync-waits on single-wait-slot instructions onto
    EventSemaphore instructions inserted immediately before them on the same
    engine queue (semantically identical)."""
    import orjson

    m = orjson.loads(data)
    cnt = 0
    for fn in m.get("functions", []):
        for bb in fn.get("blocks", []):
            insts = bb.get("instructions")
            if not insts:
                continue
            out = []
            changed = False
            for ins in insts:
                # walrus requires 5-dim input APs on Pool; bass emits the raw
                # (optimized) AP, so pad singleton dims after the partition dim
                if ins.get("opcode") == "Pool":
                    for a in ins.get("ins", []):
                        ap = a.get("ap")
                        if ap is not None and len(ap) < 5:
                            pad = [[1, 1]] * (5 - len(ap))
                            a["ap"] = [ap[0]] + pad + list(ap[1:])
                            changed = True
                si = ins.get("sync_info")
                if (
                    ins.get("opcode") not in _SPLIT_WAIT_SKIP
                    and si
                    and len(si.get("on_wait") or []) > 1
                ):
                    waits = si["on_wait"]
                    for w in waits[:-1]:
                        out.append({
                            "name": f"EVW-{cnt}",
                            "opcode": "EventSemaphore",
                            "engine": ins["engine"],
                            "ins": [],
                            "outs": [],
                            "debug": ins.get("debug", 0),
                            "sync_info": {"on_wait": [w], "on_update": []},
                        })
                        cnt += 1
                    si["on_wait"] = [waits[-1]]
                    changed = True
                out.append(ins)
            if changed:
                bb["instructions"] = out
    return orjson.dumps(m)


_CACHE: dict = {}


def _get_nc():
    if "nc" not in _CACHE:
        nc = build_nc()
        orig = nc.to_json_bytes
        nc.to_json_bytes = lambda: _split_matmul_waits_json(orig())
        _CACHE["nc"] = nc
    return _CACHE["nc"]


def kernel(x, w1, w2, trace: bool = False, **run_kwargs):
    x = np.ascontiguousarray(np.asarray(x, dtype=np.float32))
    assert x.shape == (B, C, H, W)
    w1t, w2l, mask3 = host_prep(w1, w2)

    nc = _get_nc()
    in_maps = [
        {"x": x[i * BPC:(i + 1) * BPC], "w1t": w1t, "w2l": w2l, "mask3": mask3}
        for i in range(NCORES)
    ]
    res = run_bass_kernel_spmd(
        nc, in_maps, core_ids=list(range(NCORES)), trace=trace, **run_kwargs
    )
    _CACHE["last_results"] = res
    out = np.concatenate([res.results[i]["out"] for i in range(NCORES)], axis=0)
    return out
